# revision 5
# baseline (speedup 1.0000x reference)
"""DeepGraphInfomax cluster kernel for Trainium2 (8 NeuronCores, Bass/Tile).

Computes, for z [N, H] and comm_ids [N] (K communities):
    zn   = z / ||z||_row                      [N, H]
    mu   = segment_mean(zn, comm_ids, K)      [K, H]
    dist = zn @ mu.T                          [N, K]
    r    = softmax(TEMP * dist, axis=1)       [N, K]

Sharding: nodes split evenly across 8 cores. Per core, pass 1 streams z,
computes zn (written out) and one-hot segment sums via PE matmuls into PSUM;
the [K, H+1] sums+counts are AllReduced across cores; pass 2 re-streams zn,
transposes per 128-node block on PE, computes dist = zn @ mu.T, then softmax.
"""
import os
import sys

sys.path.insert(0, "/opt/trn_rl_repo")

import numpy as np
from contextlib import ExitStack

from concourse import bass, mybir, tile
from concourse.bass_utils import run_bass_kernel_spmd

F32 = mybir.dt.float32
AF = mybir.ActivationFunctionType
ALU = mybir.AluOpType
AX = mybir.AxisListType

N_CORES = 8
N, H, K = 1_000_000, 128, 64
TEMP = 30.0
NPC = N // N_CORES          # 125000 nodes per core
B = 8                       # node rows per partition per macro tile
MACRO = 128 * B             # 1024 nodes per macro tile
NM = NPC // MACRO           # 122 full macros
TAIL = NPC - NM * MACRO     # 72 remaining nodes
NCID = NM * B + 1           # cid/rinv columns (last col = tail)

# ---------------------------------------------------------------------------
# Workaround: this walrus build accepts at most ONE sync-wait command per
# instruction, while Tile freely attaches several.  After scheduling, move
# every excess wait onto its own same-engine NoOp placed just before the
# instruction — semantically identical (the engine stalls at the NoOps, then
# at the instruction).
def split_excess_waits(nc):
    n_split = 0
    for f in nc.m.functions:
        for blk in f.blocks:
            insts = list(blk.instructions)
            out = []
            changed = False
            for ins in insts:
                si = ins.sync_info
                if si is not None and si.on_wait and len(si.on_wait) > 1:
                    waits = list(si.on_wait)
                    for w in waits[:-1]:
                        nop = mybir.InstNoOp(
                            name=f"wsplit-{nc.next_id()}", ins=[], outs=[])
                        nop.engine = ins.engine
                        nop.sync_info = mybir.SyncInfo(on_wait=[w], on_update=[])
                        nc.register_instruction(nop)
                        out.append(nop)
                        n_split += 1
                    ins.sync_info = mybir.SyncInfo(
                        on_wait=[waits[-1]], on_update=list(si.on_update))
                    changed = True
                out.append(ins)
            if changed:
                blk.instructions = out
    return n_split
# ---------------------------------------------------------------------------

# Engine picks for the two 8-op groups in pass 1 (swap if profile says so).
ONEHOT_ENGINE = os.environ.get("ONEHOT_ENGINE", "vector")
ZNMUL_ENGINE = os.environ.get("ZNMUL_ENGINE", "gpsimd")


def build(npc=NPC):
    nm = npc // MACRO
    tail = npc - nm * MACRO
    ncid = nm * B + (1 if tail else 0)

    nc = bass.Bass("TRN2", target_bir_lowering=False, debug=False,
                   num_devices=N_CORES)
    z = nc.dram_tensor("z", [npc, H], F32, kind="ExternalInput").ap()
    cid_t = nc.dram_tensor("cid_t", [128, ncid], F32, kind="ExternalInput").ap()
    iota2d = nc.dram_tensor("iota2d", [128, K], F32, kind="ExternalInput").ap()
    id128 = nc.dram_tensor("id128", [128, 128], F32, kind="ExternalInput").ap()
    zn = nc.dram_tensor("zn", [npc, H], F32, kind="ExternalOutput").ap()
    mu = nc.dram_tensor("mu", [K, H], F32, kind="ExternalOutput").ap()
    r = nc.dram_tensor("r", [npc, K], F32, kind="ExternalOutput").ap()
    dist = nc.dram_tensor("dist", [npc, K], F32, kind="ExternalOutput").ap()

    onehot_eng = getattr(nc, ONEHOT_ENGINE)
    znmul_eng = getattr(nc, ZNMUL_ENGINE)

    with tile.TileContext(nc) as tc, ExitStack() as ctx:
        const = ctx.enter_context(tc.tile_pool(name="const", bufs=1))
        p_zt = ctx.enter_context(tc.tile_pool(name="zt", bufs=3))
        p_zn = ctx.enter_context(tc.tile_pool(name="znsb", bufs=3))
        p_scr = ctx.enter_context(tc.tile_pool(name="scr", bufs=2))
        p_small = ctx.enter_context(tc.tile_pool(name="small", bufs=3))
        p_ohw = ctx.enter_context(tc.tile_pool(name="ohw", bufs=2))
        p_acc = ctx.enter_context(tc.tile_pool(name="acc", bufs=1, space="PSUM"))
        p_dram = ctx.enter_context(tc.tile_pool(name="dram", bufs=1, space="DRAM"))

        iota_sb = const.tile([128, K], F32)
        nc.sync.dma_start(iota_sb[:], iota2d)
        id_sb = const.tile([128, 128], F32)
        nc.sync.dma_start(id_sb[:], id128)
        cid_sb = const.tile([128, ncid], F32)
        nc.sync.dma_start(cid_sb[:], cid_t)

        acc = p_acc.tile([K, H + 1], F32)   # [:, :H] sums, [:, H] counts

        n_mms = nm * B + (1 if tail else 0)
        mm_i = 0

        # ----------------- pass 1: normalize + segment sums -----------------
        for m in range(nm):
            z_m = z[m * MACRO:(m + 1) * MACRO, :].rearrange("(p j) h -> p j h", j=B)
            zn_m = zn[m * MACRO:(m + 1) * MACRO, :].rearrange("(p j) h -> p j h", j=B)

            zt = p_zt.tile([128, B, H], F32, tag="zt")
            nc.sync.dma_start(zt[:], z_m)

            s2 = p_small.tile([128, B], F32, tag="s2")
            for j in range(B):
                scr = p_scr.tile([128, H], F32, tag="scr")
                nc.scalar.activation(scr[:], zt[:, j, :], AF.Square,
                                     accum_out=s2[:, j:j + 1])
            norm = p_small.tile([128, B], F32, tag="norm")
            nc.scalar.activation(norm[:], s2[:], AF.Sqrt)
            rinv = p_small.tile([128, B], F32, tag="rinv")
            nc.vector.reciprocal(rinv[:], norm[:])

            znt = p_zn.tile([128, B, H + 4], F32, tag="znt")
            for j in range(B):
                znmul_eng.tensor_scalar(znt[:, j, 0:H], zt[:, j, :],
                                        rinv[:, j:j + 1], None, op0=ALU.mult)
            nc.vector.memset(znt[:, :, H], 1.0)
            nc.sync.dma_start(zn_m, znt[:, :, 0:H])

            ohw = p_ohw.tile([128, B, K], F32, tag="ohw")
            for j in range(B):
                onehot_eng.tensor_scalar(ohw[:, j, :], iota_sb[:],
                                         cid_sb[:, m * B + j:m * B + j + 1],
                                         None, op0=ALU.is_equal)
            for j in range(B):
                nc.tensor.matmul(acc[:], ohw[:, j, :], znt[:, j, 0:H + 1],
                                 start=(mm_i == 0), stop=(mm_i == n_mms - 1))
                mm_i += 1

        if tail:
            t0 = nm * MACRO
            zt = p_zt.tile([tail, H], F32, tag="zt")
            nc.sync.dma_start(zt[:], z[t0:t0 + tail, :])
            s2 = p_small.tile([tail, 1], F32, tag="s2")
            scr = p_scr.tile([tail, H], F32, tag="scr")
            nc.scalar.activation(scr[:], zt[:], AF.Square, accum_out=s2[:])
            norm = p_small.tile([tail, 1], F32, tag="norm")
            nc.scalar.activation(norm[:], s2[:], AF.Sqrt)
            rinv = p_small.tile([tail, 1], F32, tag="rinv")
            nc.vector.reciprocal(rinv[:], norm[:])
            znt = p_zn.tile([tail, H + 4], F32, tag="znt")
            znmul_eng.tensor_scalar(znt[:, 0:H], zt[:], rinv[:], None, op0=ALU.mult)
            nc.vector.memset(znt[:, H:H + 1], 1.0)
            nc.sync.dma_start(zn[t0:t0 + tail, :], znt[:, 0:H])
            ohw = p_ohw.tile([tail, K], F32, tag="ohw")
            onehot_eng.tensor_scalar(ohw[:], iota_sb[0:tail, :],
                                     cid_sb[0:tail, ncid - 1:ncid],
                                     None, op0=ALU.is_equal)
            nc.tensor.matmul(acc[:], ohw[:], znt[:, 0:H + 1],
                             start=(mm_i == 0), stop=True)
            mm_i += 1

        # ----------------- all-reduce sums+counts, compute mu ---------------
        acc_sb = p_small.tile([K, H + 1], F32, tag="acc_sb")
        nc.scalar.copy(acc_sb[:], acc[:])
        cc_in = p_dram.tile([K, H + 1], F32)
        cc_out = p_dram.tile([K, H + 1], F32)
        nc.sync.dma_start(cc_in[:], acc_sb[:])
        nc.gpsimd.collective_compute(
            "AllReduce", ALU.add,
            replica_groups=[list(range(N_CORES))],
            ins=[cc_in.opt()], outs=[cc_out.opt()],
        )
        red_sb = p_small.tile([K, H + 1], F32, tag="red_sb")
        nc.sync.dma_start(red_sb[:], cc_out[:])

        cinv = p_small.tile([K, 1], F32, tag="cinv")
        nc.vector.reciprocal(cinv[:], red_sb[:, H:H + 1])
        mu_sb = p_small.tile([K, H], F32, tag="mu_sb")
        nc.vector.tensor_scalar(mu_sb[:], red_sb[:, 0:H], cinv[:], None,
                                op0=ALU.mult)
        nc.sync.dma_start(mu, mu_sb[:])

        p_mt = ctx.enter_context(tc.tile_pool(name="mt", bufs=1, space="PSUM"))
        muT_ps = p_mt.tile([H, K], F32)
        nc.tensor.matmul(muT_ps[:], mu_sb[:], id_sb[0:K, 0:K], is_transpose=True)
        muT_sb = p_small.tile([H, K], F32, tag="muT_sb")
        nc.scalar.copy(muT_sb[:], muT_ps[:])

        # ----------------- pass 2: dist = zn @ mu.T, softmax ----------------
        p_znb = ctx.enter_context(tc.tile_pool(name="znb", bufs=3))
        p_znT_ps = ctx.enter_context(tc.tile_pool(name="znT_ps", bufs=2, space="PSUM"))
        p_znT = ctx.enter_context(tc.tile_pool(name="znT", bufs=2))
        p_dist_ps = ctx.enter_context(tc.tile_pool(name="dist_ps", bufs=2, space="PSUM"))
        p_out = ctx.enter_context(tc.tile_pool(name="out", bufs=3))

        for m in range(nm):
            zn_m = zn[m * MACRO:(m + 1) * MACRO, :].rearrange("(p j) h -> p j h", j=B)
            dist_m = dist[m * MACRO:(m + 1) * MACRO, :].rearrange("(p j) k -> p j k", j=B)
            r_m = r[m * MACRO:(m + 1) * MACRO, :].rearrange("(p j) k -> p j k", j=B)

            znb = p_znb.tile([128, B, H], F32, tag="znb")
            nc.sync.dma_start(znb[:], zn_m)

            znT_ps = p_znT_ps.tile([128, B * 128], F32, tag="znT_ps")
            for j in range(B):
                nc.tensor.transpose(znT_ps[:, j * 128:(j + 1) * 128],
                                    znb[:, j, :], id_sb[:])
            znT = p_znT.tile([128, B * 128], F32, tag="znT")
            nc.scalar.copy(znT[:], znT_ps[:])

            dist_ps = p_dist_ps.tile([128, B, K], F32, tag="dist_ps")
            for j in range(B):
                nc.tensor.matmul(dist_ps[:, j, :],
                                 znT[:, j * 128:(j + 1) * 128], muT_sb[:])
            dist_sb = p_out.tile([128, B, K], F32, tag="dist_sb")
            nc.scalar.copy(dist_sb[:], dist_ps[:])
            nc.sync.dma_start(dist_m, dist_sb[:])

            exp_sb = p_out.tile([128, B, K], F32, tag="exp_sb")
            nc.scalar.activation(exp_sb[:], dist_sb[:], AF.Exp, scale=TEMP)
            es = p_small.tile([128, B], F32, tag="es")
            nc.vector.reduce_sum(es[:], exp_sb[:], axis=AX.X)
            esr = p_small.tile([128, B], F32, tag="esr")
            nc.vector.reciprocal(esr[:], es[:])
            r_sb = p_out.tile([128, B, K], F32, tag="r_sb")
            esr_b = esr.rearrange("p (j o) -> p j o", o=1).to_broadcast((128, B, K))
            nc.vector.tensor_mul(r_sb[:], exp_sb[:], esr_b)
            nc.sync.dma_start(r_m, r_sb[:])

        if tail:
            t0 = nm * MACRO
            znb = p_znb.tile([tail, H], F32, tag="znb")
            nc.sync.dma_start(znb[:], zn[t0:t0 + tail, :])
            znT_ps = p_znT_ps.tile([H, tail], F32, tag="znT_ps")
            nc.tensor.matmul(znT_ps[:], znb[:], id_sb[0:tail, 0:tail],
                             is_transpose=True)
            znT = p_znT.tile([H, tail], F32, tag="znT")
            nc.scalar.copy(znT[:], znT_ps[:])
            dist_ps = p_dist_ps.tile([tail, K], F32, tag="dist_ps")
            nc.tensor.matmul(dist_ps[:], znT[:], muT_sb[:])
            dist_sb = p_out.tile([tail, K], F32, tag="dist_sb")
            nc.scalar.copy(dist_sb[:], dist_ps[:])
            nc.sync.dma_start(dist[t0:t0 + tail, :], dist_sb[:])
            exp_sb = p_out.tile([tail, K], F32, tag="exp_sb")
            nc.scalar.activation(exp_sb[:], dist_sb[:], AF.Exp, scale=TEMP)
            es = p_small.tile([tail, 1], F32, tag="es")
            nc.vector.reduce_sum(es[:], exp_sb[:], axis=AX.X)
            esr = p_small.tile([tail, 1], F32, tag="esr")
            nc.vector.reciprocal(esr[:], es[:])
            r_sb = p_out.tile([tail, K], F32, tag="r_sb")
            nc.vector.tensor_mul(r_sb[:], exp_sb[:], esr.to_broadcast((tail, K)))
            nc.sync.dma_start(r[t0:t0 + tail, :], r_sb[:])

    split_excess_waits(nc)
    return nc


def make_cid_t(cid_f32, npc):
    """[128, ncid] layout: col m*B+j holds cid[m*MACRO + p*B + j] at row p;
    last col holds the tail (rows 0..tail-1)."""
    nm = npc // MACRO
    tail = npc - nm * MACRO
    ncid = nm * B + (1 if tail else 0)
    out = np.zeros((128, ncid), dtype=np.float32)
    main = cid_f32[:nm * MACRO].reshape(nm, 128, B)        # [m, p, j]
    out[:, :nm * B] = main.transpose(1, 0, 2).reshape(128, nm * B)
    if tail:
        out[:tail, ncid - 1] = cid_f32[nm * MACRO:]
    return out


_cached = {}


def _get_nc(npc):
    if npc not in _cached:
        _cached[npc] = build(npc)
    return _cached[npc]


def kernel(z, comm_ids, trace=False):
    z = np.ascontiguousarray(np.asarray(z, dtype=np.float32))
    cid = np.asarray(comm_ids)
    assert z.shape == (N, H) and cid.shape == (N,)
    cid_f32 = cid.astype(np.float32)

    iota2d = np.broadcast_to(np.arange(K, dtype=np.float32), (128, K)).copy()
    id128 = np.eye(128, dtype=np.float32)

    nc = _get_nc(NPC)
    in_maps = []
    for c in range(N_CORES):
        lo = c * NPC
        in_maps.append({
            "z": z[lo:lo + NPC],
            "cid_t": make_cid_t(cid_f32[lo:lo + NPC], NPC),
            "iota2d": iota2d,
            "id128": id128,
        })
    res = run_bass_kernel_spmd(nc, in_maps, core_ids=list(range(N_CORES)),
                               trace=trace)
    zn = np.concatenate([res.results[c]["zn"] for c in range(N_CORES)], axis=0)
    mu = res.results[0]["mu"]
    r_ = np.concatenate([res.results[c]["r"] for c in range(N_CORES)], axis=0)
    d_ = np.concatenate([res.results[c]["dist"] for c in range(N_CORES)], axis=0)
    if trace:
        kernel.last_results = res
    return zn, mu, r_, d_


# revision 11
# speedup vs baseline: 2.3509x; 2.3509x over previous
"""DeepGraphInfomax cluster kernel for Trainium2 (8 NeuronCores, Bass/Tile).

Computes, for z [N, H] and comm_ids [N] (K communities):
    zn   = z / ||z||_row                      [N, H]
    mu   = segment_mean(zn, comm_ids, K)      [K, H]
    dist = zn @ mu.T                          [N, K]
    r    = softmax(TEMP * dist, axis=1)       [N, K]

Sharding: nodes split evenly across 8 cores. Per core, pass 1 streams z,
computes zn (written out) and one-hot segment sums via PE matmuls into PSUM;
the [K, H+1] sums+counts are AllReduced across cores; pass 2 re-streams zn,
transposes per 128-node block on PE, computes dist = zn @ mu.T, then softmax.
"""
import os
import sys

sys.path.insert(0, "/opt/trn_rl_repo")

import numpy as np
from contextlib import ExitStack

from concourse import bass, mybir, tile
from concourse.bass_utils import run_bass_kernel_spmd

F32 = mybir.dt.float32
AF = mybir.ActivationFunctionType
ALU = mybir.AluOpType
AX = mybir.AxisListType

N_CORES = 8
N, H, K = 1_000_000, 128, 64
TEMP = 30.0
NPC = N // N_CORES          # 125000 nodes per core
B = 8                       # node rows per partition per macro tile
MACRO = 128 * B             # 1024 nodes per macro tile
NM = NPC // MACRO           # 122 full macros
TAIL = NPC - NM * MACRO     # 72 remaining nodes
NCID = NM * B + 1           # cid/rinv columns (last col = tail)

# ---------------------------------------------------------------------------
# Workaround: this walrus build accepts at most ONE sync-wait command per
# instruction, while Tile freely attaches several.  After scheduling, move
# every excess wait onto its own same-engine NoOp placed just before the
# instruction — semantically identical (the engine stalls at the NoOps, then
# at the instruction).
def split_excess_waits(nc):
    n_split = 0
    for f in nc.m.functions:
        for blk in f.blocks:
            insts = list(blk.instructions)
            out = []
            changed = False
            for ins in insts:
                si = ins.sync_info
                if si is not None and si.on_wait and len(si.on_wait) > 1:
                    waits = list(si.on_wait)
                    for w in waits[:-1]:
                        nop = mybir.InstNoOp(
                            name=f"wsplit-{nc.next_id()}", ins=[], outs=[])
                        nop.engine = ins.engine
                        nop.sync_info = mybir.SyncInfo(on_wait=[w], on_update=[])
                        nc.register_instruction(nop)
                        out.append(nop)
                        n_split += 1
                    ins.sync_info = mybir.SyncInfo(
                        on_wait=[waits[-1]], on_update=list(si.on_update))
                    changed = True
                out.append(ins)
            if changed:
                blk.instructions = out
    return n_split
# ---------------------------------------------------------------------------

# How many of the 8 per-macro zn-multiply blocks go to DVE (rest go to ACT).
ZNMUL_DVE = int(os.environ.get("ZNMUL_DVE", "5"))


def build(npc=NPC):
    nm = npc // MACRO
    tail = npc - nm * MACRO
    ncid = nm * B + (1 if tail else 0)

    nc = bass.Bass("TRN2", target_bir_lowering=False, debug=False,
                   num_devices=N_CORES)
    z = nc.dram_tensor("z", [npc, H], F32, kind="ExternalInput").ap()
    cid_t = nc.dram_tensor("cid_t", [128, ncid], F32, kind="ExternalInput").ap()
    iota2d = nc.dram_tensor("iota2d", [128, K], F32, kind="ExternalInput").ap()
    id128 = nc.dram_tensor("id128", [128, 128], F32, kind="ExternalInput").ap()
    zn = nc.dram_tensor("zn", [npc, H], F32, kind="ExternalOutput").ap()
    mu = nc.dram_tensor("mu", [K, H], F32, kind="ExternalOutput").ap()
    r = nc.dram_tensor("r", [npc, K], F32, kind="ExternalOutput").ap()
    dist = nc.dram_tensor("dist", [npc, K], F32, kind="ExternalOutput").ap()

    with tile.TileContext(nc) as tc, ExitStack() as ctx:
        const = ctx.enter_context(tc.tile_pool(name="const", bufs=1))
        p_zt = ctx.enter_context(tc.tile_pool(name="zt", bufs=3))
        p_zn = ctx.enter_context(tc.tile_pool(name="znsb", bufs=3))
        p_scr = ctx.enter_context(tc.tile_pool(name="scr", bufs=2))
        p_small = ctx.enter_context(tc.tile_pool(name="small", bufs=3))
        p_ohw = ctx.enter_context(tc.tile_pool(name="ohw", bufs=2))
        p_acc = ctx.enter_context(tc.tile_pool(name="acc", bufs=1, space="PSUM"))
        p_dram = ctx.enter_context(tc.tile_pool(name="dram", bufs=1, space="DRAM"))

        iota_sb = const.tile([128, K], F32)
        nc.sync.dma_start(iota_sb[:], iota2d)
        id_sb = const.tile([128, 128], F32)
        nc.sync.dma_start(id_sb[:], id128)
        cid_sb = const.tile([128, ncid], F32)
        nc.sync.dma_start(cid_sb[:], cid_t)

        acc = p_acc.tile([K, H + 1], F32)   # [:, :H] sums, [:, H] counts

        n_mms = nm * B + (1 if tail else 0)
        mm_i = 0

        # ----------------- pass 1: normalize + segment sums -----------------
        for m in range(nm):
            z_m = z[m * MACRO:(m + 1) * MACRO, :].rearrange("(p j) h -> p j h", j=B)
            zn_m = zn[m * MACRO:(m + 1) * MACRO, :].rearrange("(p j) h -> p j h", j=B)

            zt = p_zt.tile([128, B, H], F32, tag="zt")
            nc.sync.dma_start(zt[:], z_m)

            sq = p_scr.tile([128, B, H], F32, tag="sq")
            nc.scalar.activation(sq[:], zt[:], AF.Square)
            s2 = p_small.tile([128, B], F32, tag="s2")
            nc.vector.reduce_sum(s2[:], sq[:], axis=AX.X)
            norm = p_small.tile([128, B], F32, tag="norm")
            nc.scalar.activation(norm[:], s2[:], AF.Sqrt)
            rinv = p_small.tile([128, B], F32, tag="rinv")
            nc.vector.reciprocal(rinv[:], norm[:])

            znt = p_zn.tile([128, B, H + 4], F32, tag="znt")
            for j in range(B):
                if j < ZNMUL_DVE:
                    nc.vector.tensor_scalar(znt[:, j, 0:H], zt[:, j, :],
                                            rinv[:, j:j + 1], None, op0=ALU.mult)
                else:
                    nc.scalar.mul(znt[:, j, 0:H], zt[:, j, :], rinv[:, j:j + 1])
            nc.vector.memset(znt[:, :, H], 1.0)
            nc.sync.dma_start(zn_m, znt[:, :, 0:H])

            ohw = p_ohw.tile([128, B, K], F32, tag="ohw")
            iota_b = iota_sb[:, :].rearrange(
                "p (o k) -> p o k", o=1).to_broadcast((128, B, K))
            cid_b = cid_sb[:, m * B:(m + 1) * B].rearrange(
                "p (j o) -> p j o", o=1).to_broadcast((128, B, K))
            nc.vector.tensor_tensor(ohw[:], iota_b, cid_b, op=ALU.is_equal)
            for j in range(B):
                nc.tensor.matmul(acc[:], ohw[:, j, :], znt[:, j, 0:H + 1],
                                 start=(mm_i == 0), stop=(mm_i == n_mms - 1))
                mm_i += 1

        if tail:
            t0 = nm * MACRO
            zt = p_zt.tile([tail, H], F32, tag="zt")
            nc.sync.dma_start(zt[:], z[t0:t0 + tail, :])
            s2 = p_small.tile([tail, 1], F32, tag="s2")
            scr = p_scr.tile([tail, H], F32, tag="scr")
            nc.scalar.activation(scr[:], zt[:], AF.Square, accum_out=s2[:])
            norm = p_small.tile([tail, 1], F32, tag="norm")
            nc.scalar.activation(norm[:], s2[:], AF.Sqrt)
            rinv = p_small.tile([tail, 1], F32, tag="rinv")
            nc.vector.reciprocal(rinv[:], norm[:])
            znt = p_zn.tile([tail, H + 4], F32, tag="znt")
            nc.vector.tensor_scalar(znt[:, 0:H], zt[:], rinv[:], None, op0=ALU.mult)
            nc.vector.memset(znt[:, H:H + 1], 1.0)
            nc.sync.dma_start(zn[t0:t0 + tail, :], znt[:, 0:H])
            ohw = p_ohw.tile([tail, K], F32, tag="ohw")
            nc.vector.tensor_scalar(ohw[:], iota_sb[0:tail, :],
                                    cid_sb[0:tail, ncid - 1:ncid],
                                    None, op0=ALU.is_equal)
            nc.tensor.matmul(acc[:], ohw[:], znt[:, 0:H + 1],
                             start=(mm_i == 0), stop=True)
            mm_i += 1

        # ----------------- all-reduce sums+counts, compute mu ---------------
        acc_sb = p_small.tile([K, H + 1], F32, tag="acc_sb")
        nc.scalar.copy(acc_sb[:], acc[:])
        cc_in = p_dram.tile([K, H + 1], F32)
        cc_out = p_dram.tile([K, H + 1], F32)
        nc.sync.dma_start(cc_in[:], acc_sb[:])
        nc.gpsimd.collective_compute(
            "AllReduce", ALU.add,
            replica_groups=[list(range(N_CORES))],
            ins=[cc_in.opt()], outs=[cc_out.opt()],
        )
        red_sb = p_small.tile([K, H + 1], F32, tag="red_sb")
        nc.sync.dma_start(red_sb[:], cc_out[:])

        cinv = p_small.tile([K, 1], F32, tag="cinv")
        nc.vector.reciprocal(cinv[:], red_sb[:, H:H + 1])
        mu_sb = p_small.tile([K, H], F32, tag="mu_sb")
        nc.vector.tensor_scalar(mu_sb[:], red_sb[:, 0:H], cinv[:], None,
                                op0=ALU.mult)
        nc.sync.dma_start(mu, mu_sb[:])

        p_mt = ctx.enter_context(tc.tile_pool(name="mt", bufs=1, space="PSUM"))
        muT_ps = p_mt.tile([H, K], F32)
        nc.tensor.matmul(muT_ps[:], mu_sb[:], id_sb[0:K, 0:K], is_transpose=True)
        muT_sb = p_small.tile([H, K], F32, tag="muT_sb")
        nc.scalar.copy(muT_sb[:], muT_ps[:])

        # ----------------- pass 2: dist = zn @ mu.T, softmax ----------------
        p_znb = ctx.enter_context(tc.tile_pool(name="znb", bufs=4))
        p_znT_ps = ctx.enter_context(tc.tile_pool(name="znT_ps", bufs=2, space="PSUM"))
        p_znT = ctx.enter_context(tc.tile_pool(name="znT", bufs=2))
        p_dist_ps = ctx.enter_context(tc.tile_pool(name="dist_ps", bufs=2, space="PSUM"))
        p_out = ctx.enter_context(tc.tile_pool(name="out", bufs=4))

        for m in range(nm):
            zn_m = zn[m * MACRO:(m + 1) * MACRO, :].rearrange("(p j) h -> p j h", j=B)
            dist_m = dist[m * MACRO:(m + 1) * MACRO, :].rearrange("(p j) k -> p j k", j=B)
            r_m = r[m * MACRO:(m + 1) * MACRO, :].rearrange("(p j) k -> p j k", j=B)

            znb = p_znb.tile([128, B, H], F32, tag="znb")
            nc.sync.dma_start(znb[:], zn_m)

            znT_ps = p_znT_ps.tile([128, B * 128], F32, tag="znT_ps")
            for j in range(B):
                nc.tensor.transpose(znT_ps[:, j * 128:(j + 1) * 128],
                                    znb[:, j, :], id_sb[:])
            znT = p_znT.tile([128, B * 128], F32, tag="znT")
            nc.scalar.copy(znT[:], znT_ps[:])

            dist_ps = p_dist_ps.tile([128, B, K], F32, tag="dist_ps")
            for j in range(B):
                nc.tensor.matmul(dist_ps[:, j, :],
                                 znT[:, j * 128:(j + 1) * 128], muT_sb[:])
            dist_sb = p_out.tile([128, B, K], F32, tag="dist_sb")
            nc.scalar.copy(dist_sb[:], dist_ps[:])
            nc.sync.dma_start(dist_m, dist_sb[:])

            exp_sb = p_out.tile([128, B, K], F32, tag="exp_sb")
            nc.scalar.activation(exp_sb[:], dist_sb[:], AF.Exp, scale=TEMP)
            es = p_small.tile([128, B], F32, tag="es")
            nc.vector.reduce_sum(es[:], exp_sb[:], axis=AX.X)
            esr = p_small.tile([128, B], F32, tag="esr")
            nc.vector.reciprocal(esr[:], es[:])
            r_sb = p_out.tile([128, B, K], F32, tag="r_sb")
            esr_b = esr.rearrange("p (j o) -> p j o", o=1).to_broadcast((128, B, K))
            nc.vector.tensor_mul(r_sb[:], exp_sb[:], esr_b)
            nc.sync.dma_start(r_m, r_sb[:])

        if tail:
            t0 = nm * MACRO
            znb = p_znb.tile([tail, H], F32, tag="znb")
            nc.sync.dma_start(znb[:], zn[t0:t0 + tail, :])
            znT_ps = p_znT_ps.tile([H, tail], F32, tag="znT_ps")
            nc.tensor.matmul(znT_ps[:], znb[:], id_sb[0:tail, 0:tail],
                             is_transpose=True)
            znT = p_znT.tile([H, tail], F32, tag="znT")
            nc.scalar.copy(znT[:], znT_ps[:])
            dist_ps = p_dist_ps.tile([tail, K], F32, tag="dist_ps")
            nc.tensor.matmul(dist_ps[:], znT[:], muT_sb[:])
            dist_sb = p_out.tile([tail, K], F32, tag="dist_sb")
            nc.scalar.copy(dist_sb[:], dist_ps[:])
            nc.sync.dma_start(dist[t0:t0 + tail, :], dist_sb[:])
            exp_sb = p_out.tile([tail, K], F32, tag="exp_sb")
            nc.scalar.activation(exp_sb[:], dist_sb[:], AF.Exp, scale=TEMP)
            es = p_small.tile([tail, 1], F32, tag="es")
            nc.vector.reduce_sum(es[:], exp_sb[:], axis=AX.X)
            esr = p_small.tile([tail, 1], F32, tag="esr")
            nc.vector.reciprocal(esr[:], es[:])
            r_sb = p_out.tile([tail, K], F32, tag="r_sb")
            nc.vector.tensor_mul(r_sb[:], exp_sb[:], esr.to_broadcast((tail, K)))
            nc.sync.dma_start(r[t0:t0 + tail, :], r_sb[:])

    split_excess_waits(nc)
    return nc


def make_cid_t(cid_f32, npc):
    """[128, ncid] layout: col m*B+j holds cid[m*MACRO + p*B + j] at row p;
    last col holds the tail (rows 0..tail-1)."""
    nm = npc // MACRO
    tail = npc - nm * MACRO
    ncid = nm * B + (1 if tail else 0)
    out = np.zeros((128, ncid), dtype=np.float32)
    main = cid_f32[:nm * MACRO].reshape(nm, 128, B)        # [m, p, j]
    out[:, :nm * B] = main.transpose(1, 0, 2).reshape(128, nm * B)
    if tail:
        out[:tail, ncid - 1] = cid_f32[nm * MACRO:]
    return out


_cached = {}


def _get_nc(npc):
    if npc not in _cached:
        _cached[npc] = build(npc)
    return _cached[npc]


def kernel(z, comm_ids, trace=False):
    z = np.ascontiguousarray(np.asarray(z, dtype=np.float32))
    cid = np.asarray(comm_ids)
    assert z.shape == (N, H) and cid.shape == (N,)
    cid_f32 = cid.astype(np.float32)

    iota2d = np.broadcast_to(np.arange(K, dtype=np.float32), (128, K)).copy()
    id128 = np.eye(128, dtype=np.float32)

    nc = _get_nc(NPC)
    in_maps = []
    for c in range(N_CORES):
        lo = c * NPC
        in_maps.append({
            "z": z[lo:lo + NPC],
            "cid_t": make_cid_t(cid_f32[lo:lo + NPC], NPC),
            "iota2d": iota2d,
            "id128": id128,
        })
    res = run_bass_kernel_spmd(nc, in_maps, core_ids=list(range(N_CORES)),
                               trace=trace)
    zn = np.concatenate([res.results[c]["zn"] for c in range(N_CORES)], axis=0)
    mu = res.results[0]["mu"]
    r_ = np.concatenate([res.results[c]["r"] for c in range(N_CORES)], axis=0)
    d_ = np.concatenate([res.results[c]["dist"] for c in range(N_CORES)], axis=0)
    if trace:
        kernel.last_results = res
    return zn, mu, r_, d_


# revision 12
# speedup vs baseline: 2.7411x; 1.1660x over previous
"""DeepGraphInfomax cluster kernel for Trainium2 (8 NeuronCores, Bass/Tile).

Computes, for z [N, H] and comm_ids [N] (K communities):
    zn   = z / ||z||_row                      [N, H]
    mu   = segment_mean(zn, comm_ids, K)      [K, H]
    dist = zn @ mu.T                          [N, K]
    r    = softmax(TEMP * dist, axis=1)       [N, K]

Sharding: nodes split evenly across 8 cores. Per core, pass 1 streams z,
computes zn (written out) and one-hot segment sums via PE matmuls into PSUM;
the [K, H+1] sums+counts are AllReduced across cores; pass 2 re-streams zn,
transposes per 128-node block on PE, computes dist = zn @ mu.T, then softmax.
"""
import os
import sys

sys.path.insert(0, "/opt/trn_rl_repo")

import numpy as np
from contextlib import ExitStack

from concourse import bass, mybir, tile
from concourse.bass_utils import run_bass_kernel_spmd

F32 = mybir.dt.float32
AF = mybir.ActivationFunctionType
ALU = mybir.AluOpType
AX = mybir.AxisListType

N_CORES = 8
N, H, K = 1_000_000, 128, 64
TEMP = 30.0
NPC = N // N_CORES          # 125000 nodes per core
B = 8                       # node rows per partition per macro tile
MACRO = 128 * B             # 1024 nodes per macro tile
NM = NPC // MACRO           # 122 full macros
TAIL = NPC - NM * MACRO     # 72 remaining nodes
NCID = NM * B + 1           # cid/rinv columns (last col = tail)

# ---------------------------------------------------------------------------
# Workaround: this walrus build accepts at most ONE sync-wait command per
# instruction, while Tile freely attaches several.  After scheduling, move
# every excess wait onto its own same-engine NoOp placed just before the
# instruction — semantically identical (the engine stalls at the NoOps, then
# at the instruction).
def split_excess_waits(nc):
    n_split = 0
    for f in nc.m.functions:
        for blk in f.blocks:
            insts = list(blk.instructions)
            out = []
            changed = False
            for ins in insts:
                si = ins.sync_info
                if si is not None and si.on_wait and len(si.on_wait) > 1:
                    waits = list(si.on_wait)
                    for w in waits[:-1]:
                        nop = mybir.InstNoOp(
                            name=f"wsplit-{nc.next_id()}", ins=[], outs=[])
                        nop.engine = ins.engine
                        nop.sync_info = mybir.SyncInfo(on_wait=[w], on_update=[])
                        nc.register_instruction(nop)
                        out.append(nop)
                        n_split += 1
                    ins.sync_info = mybir.SyncInfo(
                        on_wait=[waits[-1]], on_update=list(si.on_update))
                    changed = True
                out.append(ins)
            if changed:
                blk.instructions = out
    return n_split
# ---------------------------------------------------------------------------

# How many of the 8 per-macro zn-multiply blocks go to DVE (rest go to ACT).
ZNMUL_DVE = int(os.environ.get("ZNMUL_DVE", "5"))


def build(npc=NPC):
    nm = npc // MACRO
    tail = npc - nm * MACRO
    ncid = nm * B + (1 if tail else 0)

    nc = bass.Bass("TRN2", target_bir_lowering=False, debug=False,
                   num_devices=N_CORES)
    z = nc.dram_tensor("z", [npc, H], F32, kind="ExternalInput").ap()
    cid_t = nc.dram_tensor("cid_t", [128, ncid], F32, kind="ExternalInput").ap()
    iota2d = nc.dram_tensor("iota2d", [128, K], F32, kind="ExternalInput").ap()
    id128 = nc.dram_tensor("id128", [128, 128], F32, kind="ExternalInput").ap()
    zn = nc.dram_tensor("zn", [npc, H], F32, kind="ExternalOutput").ap()
    mu = nc.dram_tensor("mu", [K, H], F32, kind="ExternalOutput").ap()
    r = nc.dram_tensor("r", [npc, K], F32, kind="ExternalOutput").ap()
    dist = nc.dram_tensor("dist", [npc, K], F32, kind="ExternalOutput").ap()

    with tile.TileContext(nc) as tc, ExitStack() as ctx:
        const = ctx.enter_context(tc.tile_pool(name="const", bufs=1))
        p_zt = ctx.enter_context(tc.tile_pool(name="zt", bufs=3))
        p_zn = ctx.enter_context(tc.tile_pool(name="znsb", bufs=3))
        p_scr = ctx.enter_context(tc.tile_pool(name="scr", bufs=2))
        p_small = ctx.enter_context(tc.tile_pool(name="small", bufs=3))
        p_ohw = ctx.enter_context(tc.tile_pool(name="ohw", bufs=2))
        p_acc = ctx.enter_context(tc.tile_pool(name="acc", bufs=1, space="PSUM"))
        p_dram = ctx.enter_context(tc.tile_pool(name="dram", bufs=1, space="DRAM"))

        iota_sb = const.tile([128, K], F32)
        nc.sync.dma_start(iota_sb[:], iota2d)
        id_sb = const.tile([128, 128], F32)
        nc.sync.dma_start(id_sb[:], id128)
        cid_sb = const.tile([128, ncid], F32)
        nc.sync.dma_start(cid_sb[:], cid_t)

        acc = p_acc.tile([K, H + 1], F32)   # [:, :H] sums, [:, H] counts

        n_mms = nm * B + (1 if tail else 0)
        mm_i = 0

        # ----------------- pass 1: normalize + segment sums -----------------
        for m in range(nm):
            z_m = z[m * MACRO:(m + 1) * MACRO, :].rearrange("(p j) h -> p j h", j=B)
            zn_m = zn[m * MACRO:(m + 1) * MACRO, :].rearrange("(p j) h -> p j h", j=B)

            zt = p_zt.tile([128, B, H], F32, tag="zt")
            nc.sync.dma_start(zt[:], z_m)

            sq = p_scr.tile([128, B, H], F32, tag="sq")
            nc.scalar.activation(sq[:], zt[:], AF.Square)
            s2 = p_small.tile([128, B], F32, tag="s2")
            nc.vector.reduce_sum(s2[:], sq[:], axis=AX.X)
            norm = p_small.tile([128, B], F32, tag="norm")
            nc.scalar.activation(norm[:], s2[:], AF.Sqrt)
            rinv = p_small.tile([128, B], F32, tag="rinv")
            nc.vector.reciprocal(rinv[:], norm[:])

            znt = p_zn.tile([128, B, H + 4], F32, tag="znt")
            for j in range(B):
                if j < ZNMUL_DVE:
                    nc.vector.tensor_scalar(znt[:, j, 0:H], zt[:, j, :],
                                            rinv[:, j:j + 1], None, op0=ALU.mult)
                else:
                    nc.scalar.mul(znt[:, j, 0:H], zt[:, j, :], rinv[:, j:j + 1])
            nc.vector.memset(znt[:, :, H], 1.0)
            nc.sync.dma_start(zn_m, znt[:, :, 0:H])

            ohw = p_ohw.tile([128, B, K], F32, tag="ohw")
            iota_b = iota_sb[:, :].rearrange(
                "p (o k) -> p o k", o=1).to_broadcast((128, B, K))
            cid_b = cid_sb[:, m * B:(m + 1) * B].rearrange(
                "p (j o) -> p j o", o=1).to_broadcast((128, B, K))
            nc.vector.tensor_tensor(ohw[:], iota_b, cid_b, op=ALU.is_equal)
            for j in range(B):
                nc.tensor.matmul(acc[:], ohw[:, j, :], znt[:, j, 0:H + 1],
                                 start=(mm_i == 0), stop=(mm_i == n_mms - 1))
                mm_i += 1

        if tail:
            t0 = nm * MACRO
            zt = p_zt.tile([tail, H], F32, tag="zt")
            nc.sync.dma_start(zt[:], z[t0:t0 + tail, :])
            s2 = p_small.tile([tail, 1], F32, tag="s2")
            scr = p_scr.tile([tail, H], F32, tag="scr")
            nc.scalar.activation(scr[:], zt[:], AF.Square, accum_out=s2[:])
            norm = p_small.tile([tail, 1], F32, tag="norm")
            nc.scalar.activation(norm[:], s2[:], AF.Sqrt)
            rinv = p_small.tile([tail, 1], F32, tag="rinv")
            nc.vector.reciprocal(rinv[:], norm[:])
            znt = p_zn.tile([tail, H + 4], F32, tag="znt")
            nc.vector.tensor_scalar(znt[:, 0:H], zt[:], rinv[:], None, op0=ALU.mult)
            nc.vector.memset(znt[:, H:H + 1], 1.0)
            nc.sync.dma_start(zn[t0:t0 + tail, :], znt[:, 0:H])
            ohw = p_ohw.tile([tail, K], F32, tag="ohw")
            nc.vector.tensor_scalar(ohw[:], iota_sb[0:tail, :],
                                    cid_sb[0:tail, ncid - 1:ncid],
                                    None, op0=ALU.is_equal)
            nc.tensor.matmul(acc[:], ohw[:], znt[:, 0:H + 1],
                             start=(mm_i == 0), stop=True)
            mm_i += 1

        # ----------------- all-reduce sums+counts, compute mu ---------------
        acc_sb = p_small.tile([K, H + 1], F32, tag="acc_sb")
        nc.scalar.copy(acc_sb[:], acc[:])
        cc_in = p_dram.tile([K, H + 1], F32)
        cc_out = p_dram.tile([K, H + 1], F32)
        nc.sync.dma_start(cc_in[:], acc_sb[:])
        nc.gpsimd.collective_compute(
            "AllReduce", ALU.add,
            replica_groups=[list(range(N_CORES))],
            ins=[cc_in.opt()], outs=[cc_out.opt()],
        )
        red_sb = p_small.tile([K, H + 1], F32, tag="red_sb")
        nc.sync.dma_start(red_sb[:], cc_out[:])

        cinv = p_small.tile([K, 1], F32, tag="cinv")
        nc.vector.reciprocal(cinv[:], red_sb[:, H:H + 1])
        mu_sb = p_small.tile([K, H], F32, tag="mu_sb")
        nc.vector.tensor_scalar(mu_sb[:], red_sb[:, 0:H], cinv[:], None,
                                op0=ALU.mult)
        nc.sync.dma_start(mu, mu_sb[:])

        # ----------------- pass 2: dist = zn @ mu.T, softmax ----------------
        # Software pipeline: stage A (zn load + PE transpose + ACT copy) has no
        # dependency on the collective, so it is emitted PIPE macros ahead —
        # the in-order PE chews transposes while the AllReduce completes.
        PIPE = 6
        p_mt = ctx.enter_context(tc.tile_pool(name="mt", bufs=1, space="PSUM"))
        p_znb = ctx.enter_context(tc.tile_pool(name="znb", bufs=PIPE + 2))
        p_znT_ps = ctx.enter_context(tc.tile_pool(name="znT_ps", bufs=2, space="PSUM"))
        p_znT = ctx.enter_context(tc.tile_pool(name="znT", bufs=PIPE + 2))
        p_dist_ps = ctx.enter_context(tc.tile_pool(name="dist_ps", bufs=2, space="PSUM"))
        p_out = ctx.enter_context(tc.tile_pool(name="out", bufs=4))

        def stage_a(m):
            zn_m = zn[m * MACRO:(m + 1) * MACRO, :].rearrange("(p j) h -> p j h", j=B)
            znb = p_znb.tile([128, B, H], F32, tag="znb")
            nc.sync.dma_start(znb[:], zn_m)
            znT_ps = p_znT_ps.tile([128, B * 128], F32, tag="znT_ps")
            for j in range(B):
                nc.tensor.transpose(znT_ps[:, j * 128:(j + 1) * 128],
                                    znb[:, j, :], id_sb[:])
            znT = p_znT.tile([128, B * 128], F32, tag="znT")
            nc.scalar.copy(znT[:], znT_ps[:])
            return znT

        znT_q = {}
        for m in range(min(PIPE, nm)):
            znT_q[m] = stage_a(m)

        muT_ps = p_mt.tile([H, K], F32)
        nc.tensor.matmul(muT_ps[:], mu_sb[:], id_sb[0:K, 0:K], is_transpose=True)
        muT_sb = p_small.tile([H, K], F32, tag="muT_sb")
        nc.scalar.copy(muT_sb[:], muT_ps[:])

        for m in range(nm):
            if m + PIPE < nm:
                znT_q[m + PIPE] = stage_a(m + PIPE)
            znT = znT_q.pop(m)
            dist_m = dist[m * MACRO:(m + 1) * MACRO, :].rearrange("(p j) k -> p j k", j=B)
            r_m = r[m * MACRO:(m + 1) * MACRO, :].rearrange("(p j) k -> p j k", j=B)

            dist_ps = p_dist_ps.tile([128, B, K], F32, tag="dist_ps")
            for j in range(B):
                nc.tensor.matmul(dist_ps[:, j, :],
                                 znT[:, j * 128:(j + 1) * 128], muT_sb[:])
            dist_sb = p_out.tile([128, B, K], F32, tag="dist_sb")
            nc.scalar.copy(dist_sb[:], dist_ps[:])
            nc.sync.dma_start(dist_m, dist_sb[:])

            exp_sb = p_out.tile([128, B, K], F32, tag="exp_sb")
            nc.scalar.activation(exp_sb[:], dist_sb[:], AF.Exp, scale=TEMP)
            es = p_small.tile([128, B], F32, tag="es")
            nc.vector.reduce_sum(es[:], exp_sb[:], axis=AX.X)
            esr = p_small.tile([128, B], F32, tag="esr")
            nc.vector.reciprocal(esr[:], es[:])
            r_sb = p_out.tile([128, B, K], F32, tag="r_sb")
            esr_b = esr.rearrange("p (j o) -> p j o", o=1).to_broadcast((128, B, K))
            nc.vector.tensor_mul(r_sb[:], exp_sb[:], esr_b)
            nc.sync.dma_start(r_m, r_sb[:])

        if tail:
            t0 = nm * MACRO
            znb = p_znb.tile([tail, H], F32, tag="znb")
            nc.sync.dma_start(znb[:], zn[t0:t0 + tail, :])
            znT_ps = p_znT_ps.tile([H, tail], F32, tag="znT_ps")
            nc.tensor.matmul(znT_ps[:], znb[:], id_sb[0:tail, 0:tail],
                             is_transpose=True)
            znT = p_znT.tile([H, tail], F32, tag="znT")
            nc.scalar.copy(znT[:], znT_ps[:])
            dist_ps = p_dist_ps.tile([tail, K], F32, tag="dist_ps")
            nc.tensor.matmul(dist_ps[:], znT[:], muT_sb[:])
            dist_sb = p_out.tile([tail, K], F32, tag="dist_sb")
            nc.scalar.copy(dist_sb[:], dist_ps[:])
            nc.sync.dma_start(dist[t0:t0 + tail, :], dist_sb[:])
            exp_sb = p_out.tile([tail, K], F32, tag="exp_sb")
            nc.scalar.activation(exp_sb[:], dist_sb[:], AF.Exp, scale=TEMP)
            es = p_small.tile([tail, 1], F32, tag="es")
            nc.vector.reduce_sum(es[:], exp_sb[:], axis=AX.X)
            esr = p_small.tile([tail, 1], F32, tag="esr")
            nc.vector.reciprocal(esr[:], es[:])
            r_sb = p_out.tile([tail, K], F32, tag="r_sb")
            nc.vector.tensor_mul(r_sb[:], exp_sb[:], esr.to_broadcast((tail, K)))
            nc.sync.dma_start(r[t0:t0 + tail, :], r_sb[:])

    split_excess_waits(nc)
    return nc


def make_cid_t(cid_f32, npc):
    """[128, ncid] layout: col m*B+j holds cid[m*MACRO + p*B + j] at row p;
    last col holds the tail (rows 0..tail-1)."""
    nm = npc // MACRO
    tail = npc - nm * MACRO
    ncid = nm * B + (1 if tail else 0)
    out = np.zeros((128, ncid), dtype=np.float32)
    main = cid_f32[:nm * MACRO].reshape(nm, 128, B)        # [m, p, j]
    out[:, :nm * B] = main.transpose(1, 0, 2).reshape(128, nm * B)
    if tail:
        out[:tail, ncid - 1] = cid_f32[nm * MACRO:]
    return out


_cached = {}


def _get_nc(npc):
    if npc not in _cached:
        _cached[npc] = build(npc)
    return _cached[npc]


def kernel(z, comm_ids, trace=False):
    z = np.ascontiguousarray(np.asarray(z, dtype=np.float32))
    cid = np.asarray(comm_ids)
    assert z.shape == (N, H) and cid.shape == (N,)
    cid_f32 = cid.astype(np.float32)

    iota2d = np.broadcast_to(np.arange(K, dtype=np.float32), (128, K)).copy()
    id128 = np.eye(128, dtype=np.float32)

    nc = _get_nc(NPC)
    in_maps = []
    for c in range(N_CORES):
        lo = c * NPC
        in_maps.append({
            "z": z[lo:lo + NPC],
            "cid_t": make_cid_t(cid_f32[lo:lo + NPC], NPC),
            "iota2d": iota2d,
            "id128": id128,
        })
    res = run_bass_kernel_spmd(nc, in_maps, core_ids=list(range(N_CORES)),
                               trace=trace)
    zn = np.concatenate([res.results[c]["zn"] for c in range(N_CORES)], axis=0)
    mu = res.results[0]["mu"]
    r_ = np.concatenate([res.results[c]["r"] for c in range(N_CORES)], axis=0)
    d_ = np.concatenate([res.results[c]["dist"] for c in range(N_CORES)], axis=0)
    if trace:
        kernel.last_results = res
    return zn, mu, r_, d_
